# revision 1
# baseline (speedup 1.0000x reference)
"""Trainium2 Bass kernel for nn_ComplexMultiheadAttention (v4).

Problem: complex multihead attention, B=2, N=1024, D=1024, HEADS=16, d=64.
Sharding (8 cores): core = (b = c//4) x (head group hg = c%4, 4 heads).
Host sums the 4 head-group partial output projections per batch.

Key structure (v4):
- all-bf16 matmul dataflow (host pre-casts); fp32 PSUM accumulate.
- all weight DMAs issued up-front (x on Sync HWDGE queue, weights on the
  Scalar HWDGE queue), including the phase-C wy weights.
- QK row-packing: qT2 = [qr(64); qi(64)] per head, k tiles duplicated
  [kr; kr] / [ki; ki].  Two concurrent K=64 matmuls (tile_position row
  groups 0:64 / 64:128, auto-derived from base partitions) compute
  S^T for g-pairs (g0,g2) [resp (g1,g3)] in one N=512 pass -> QK PE
  time halves vs the K=128 2*S duplication trick (measured 215ns/pair).
- attention processed as (head, k-part) passes: kr-pass -> g0,g2 (both
  use vr); ki-pass -> g1,g3 (vi).  exp (ScalarE) is the pacing engine;
  a one-pass QK/exp warm-up is emitted before the v chains.
- per-(h,g) normalization: den row -> gpsimd broadcast ->
  reciprocal_approx_fast -> one [64,1024] multiply.
- phase C evacuations on ScalarE (idle there); single shared PSUM slot
  pool (3x [128,1024] tag-shared) for A chains, S tiles and C chains.
"""

import os

import numpy as np
import ml_dtypes

import concourse.mybir as mybir
import concourse.tile as tile
from concourse import bacc
from concourse.bass_utils import run_bass_kernel_spmd

P = 128
NTOK = 1024
KD = 16  # k-tiles over the stacked 2048 contraction dim
CD = 64  # dim per head
HL = 4  # heads per core
F32 = mybir.dt.float32
BF16 = mybir.dt.bfloat16
EXP = mybir.ActivationFunctionType.Exp
SCALE = float(CD) ** -0.5

_nc_cache = None


def _build():
    nc = bacc.Bacc("TRN2", target_bir_lowering=False, debug=False, num_devices=8)

    x = nc.declare_dram_parameter("x", [2048, NTOK], BF16, isOutput=False)
    wnames = ["wqr", "wqi", "wkr", "wki"]
    wd = {n: nc.declare_dram_parameter(n, [2048, 256], BF16, isOutput=False) for n in wnames}
    wv = nc.declare_dram_parameter("wv", [2048, 512], BF16, isOutput=False)
    wyr = nc.declare_dram_parameter("wyr", [512, NTOK], BF16, isOutput=False)
    wyi = nc.declare_dram_parameter("wyi", [512, NTOK], BF16, isOutput=False)
    yp = nc.declare_dram_parameter("ypart", [2, NTOK, 1024], F32, isOutput=True)
    dbg = {}
    if os.environ.get("CMHA_DEBUG"):
        for n, shp, dt in (
            ("d_qT2", [P, HL, NTOK], BF16), ("d_krT", [P, HL, NTOK], BF16),
            ("d_vhat", [P, 8, 2, HL, CD + 1], BF16), ("d_O", [P, HL, NTOK], BF16),
            ("d_pt", [P, 8, 512], BF16), ("d_oav", [CD + 1, NTOK], F32),
        ):
            dbg[n] = nc.declare_dram_parameter(n, shp, dt, isOutput=True)

    with tile.TileContext(nc) as tc:
        with (
            tc.tile_pool(name="persist", bufs=1) as pp,
            tc.tile_pool(name="ps", bufs=3, space="PSUM") as psp,
            tc.tile_pool(name="av", bufs=2, space="PSUM") as avp,
            tc.tile_pool(name="pt", bufs=3) as ptp,
            tc.tile_pool(name="oav", bufs=3) as oavp,
            tc.tile_pool(name="on", bufs=1) as onp,
            tc.tile_pool(name="nrm", bufs=1) as nrmp,
        ):
            # qT2: rows 0:64 = qr, rows 64:128 = qi (per head).
            # krT/kiT: both row-halves hold the same k part (duplicated) so a
            # packed pair of K=64 matmuls computes S^T for (g0,g2)/(g1,g3).
            qT2 = pp.tile([P, HL, NTOK], BF16, tag="qT2")
            krT = pp.tile([P, HL, NTOK], BF16, tag="krT")
            kiT = pp.tile([P, HL, NTOK], BF16, tag="kiT")
            # V with ones column appended: [tok-tile, jt, (r,i), head, 65]
            vhat = pp.tile([P, 8, 2, HL, CD + 1], BF16, tag="vhat")
            # combined attention output per head: [or(64); oi(64)] x tokens
            O = pp.tile([P, HL, NTOK], BF16, tag="O")
            wy_sb = pp.tile([P, 2, HL, NTOK], BF16, tag="wy")

            # ---- all DMAs up front: x on sync, weights on scalar (HWDGE) ----
            wts = {}
            with tc.tile_pool(name="wq", bufs=1) as wqp:
                xs = wqp.tile([P, KD, NTOK], BF16, tag="xs")
                xt = x.rearrange("(o p) m -> p o m", p=P)
                for lo, hi in ((0, 2), (2, 4), (4, 8), (8, 12), (12, 16)):
                    nc.sync.dma_start(xs[:, lo:hi, :], xt[:, lo:hi, :])
                for n in ("wqr", "wkr", "wqi", "wki"):
                    wts[n] = wqp.tile([P, KD, 256], BF16, tag="wqk", name=f"wt_{n}", bufs=2)
                    wsrc = wd[n].rearrange("(o p) m -> p o m", p=P)
                    if n in ("wqr", "wkr"):
                        nc.scalar.dma_start(wts[n][:, :, 0:128], wsrc[:, :, 0:128])
                        nc.scalar.dma_start(wts[n][:, :, 128:256], wsrc[:, :, 128:256])
                    else:
                        nc.scalar.dma_start(wts[n], wsrc)
                    if n == "wkr":
                        wvt = wqp.tile([P, KD, 512], BF16, tag="wt_v")
                        nc.scalar.dma_start(
                            wvt, wv.rearrange("(o p) m -> p o m", p=P)
                        )
                        nc.scalar.dma_start(
                            wy_sb[:, 0], wyr.rearrange("(o p) m -> p o m", p=P)
                        )
                        nc.scalar.dma_start(
                            wy_sb[:, 1], wyi.rearrange("(o p) m -> p o m", p=P)
                        )

                # ---------------- emission helpers ----------------
                def proj_chain(wn, pair):
                    # q^T/k^T = W^T x^T : lhsT = W chunk (stationary), rhs = x
                    slot = psp.tile([P, NTOK], F32, tag="s", name="slot")
                    for kt in range(KD):
                        for tch in range(2):
                            nc.tensor.matmul(
                                slot[:, tch * 512 : (tch + 1) * 512],
                                wts[wn][:, kt, pair * 128 : (pair + 1) * 128],
                                xs[:, kt, tch * 512 : (tch + 1) * 512],
                                start=(kt == 0),
                                stop=(kt == KD - 1),
                            )
                    return slot

                def emit_k_w_pair(wn, dstT, pair):
                    # duplicated layout: both row halves = the k part
                    slot = proj_chain(wn, pair)
                    hA, hB = pair * 2, pair * 2 + 1
                    nc.vector.tensor_copy(dstT[0:CD, hA, :], slot[0:CD, :])
                    nc.vector.tensor_copy(dstT[CD:P, hB, :], slot[CD:P, :])
                    nc.gpsimd.dma_start(dstT[CD:P, hA, :], dstT[0:CD, hA, :])
                    nc.gpsimd.dma_start(dstT[0:CD, hB, :], dstT[CD:P, hB, :])

                def emit_qr_w():
                    # qr -> qT2 rows 0:64.  hB arrives on psum rows 64:128;
                    # stage it in qT2[64:128, hB] (later overwritten by qi).
                    for pair in range(2):
                        slot = proj_chain("wqr", pair)
                        hA, hB = pair * 2, pair * 2 + 1
                        nc.vector.tensor_copy(qT2[0:CD, hA, :], slot[0:CD, :])
                        nc.vector.tensor_copy(qT2[CD:P, hB, :], slot[CD:P, :])
                        nc.gpsimd.dma_start(qT2[0:CD, hB, :], qT2[CD:P, hB, :])

                def emit_qi_w():
                    # qi -> qT2 rows 64:128.  Host packs wqi cols head-swapped
                    # ([qi_hB | qi_hA]) so hA lands on psum rows 64:128 direct;
                    # qi_hB goes via a scratch tile + gpsimd lift.
                    for pair in range(2):
                        slot = proj_chain("wqi", pair)
                        hA, hB = pair * 2, pair * 2 + 1
                        nc.vector.tensor_copy(qT2[CD:P, hA, :], slot[CD:P, :])
                        qsc = nrmp.tile([CD, NTOK], BF16, tag="bcd", name="qsc")
                        nc.vector.tensor_copy(qsc[:], slot[0:CD, :])
                        nc.gpsimd.dma_start(qT2[CD:P, hB, :], qsc[:])

                def emit_v(ri):
                    # v = x W : lhsT = x chunk, rhs = wv cols (256 per r/i part)
                    for tt in range(8):
                        slot = psp.tile([P, 256], F32, tag="s", name="slotv")
                        for kt in range(KD):
                            nc.tensor.matmul(
                                slot[:],
                                xs[:, kt, tt * 128 : (tt + 1) * 128],
                                wvt[:, kt, ri * 256 : (ri + 1) * 256],
                                start=(kt == 0),
                                stop=(kt == KD - 1),
                            )
                        nc.vector.tensor_copy(
                            vhat[:, tt, ri, :, 0:CD],
                            slot[:].rearrange("p (h d) -> p h d", d=CD),
                        )
                    nc.vector.memset(vhat[:, :, ri, :, CD : CD + 1], 1.0)

                def emit_pass_qk(h, kp, ic):
                    # packed pair: rows 0:64 (qr x k) and 64:128 (qi x k)
                    # run concurrently as separate row-group matmuls.
                    kT = krT if kp == 0 else kiT
                    pt0 = ptp.tile([P, 8, 512], BF16, tag="pt", name="pt0")
                    pt1 = ptp.tile([P, 8, 512], BF16, tag="pt", name="pt1")
                    for u in range(4):
                        st0 = psp.tile([P, NTOK], F32, tag="s", name="st0")
                        st1 = psp.tile([P, NTOK], F32, tag="s", name="st1")
                        for jj in range(2):
                            jt = 2 * u + jj
                            jts = slice(jt * 128, (jt + 1) * 128)
                            ics = slice(ic * 512, (ic + 1) * 512)
                            jjs = slice(jj * 512, (jj + 1) * 512)
                            nc.tensor.matmul(
                                st0[:, jjs], kT[0:CD, h, jts], qT2[0:CD, h, ics],
                                start=True, stop=True,
                            )
                            nc.tensor.matmul(
                                st1[:, jjs], kT[CD:P, h, jts], qT2[CD:P, h, ics],
                                start=True, stop=True,
                            )
                        for pt, st in ((pt0, st0), (pt1, st1)):
                            nc.scalar.activation(
                                pt[:, 2 * u : 2 * u + 2, :].rearrange("p a b -> p (a b)"),
                                st[:],
                                EXP,
                                scale=SCALE,
                            )
                    return pt0, pt1

                def emit_pass_av(h, kp, ic, pts, oavs, dens):
                    icsl = slice(ic * 512, (ic + 1) * 512)
                    for gi in range(2):
                        pt, oav, den0 = pts[gi], oavs[gi], dens[gi]
                        av = avp.tile([CD + 1, 512], F32, tag="av", name="av")
                        for jt in range(8):
                            nc.tensor.matmul(
                                av[:],
                                vhat[:, jt, kp, h, :],
                                pt[:, jt, :],
                                start=(jt == 0),
                                stop=(jt == 7),
                            )
                        if dbg and (h, kp, ic, gi) == (0, 0, 0, 0):
                            nc.sync.dma_start(dbg["d_pt"][:, :, :], pt[:])
                        nc.vector.tensor_copy(oav[:, icsl], av[:])
                        nc.gpsimd.dma_start(den0[:, icsl], oav[CD : CD + 1, icsl])

                def emit_norm(h, g, oav, den0, on_h):
                    if dbg and (h, g) == (0, 0):
                        nc.sync.dma_start(dbg["d_oav"][:, :], oav[:])
                    bcd = nrmp.tile([CD, NTOK], F32, tag="bcd", name="bcd")
                    nc.gpsimd.partition_broadcast(bcd[:], den0[:])
                    bcr = nrmp.tile([CD, NTOK], F32, tag="bcr", name="bcr")
                    nc.vector.reciprocal_approx_fast(bcr[:], bcd[:])
                    nc.vector.tensor_mul(on_h[:, g, :], oav[0:CD, :], bcr[:])

                def emit_combine(h, on_h):
                    # o_r = (o0-o3)-(o1+o2), o_i = (o0-o3)+(o1+o2)
                    sto = nrmp.tile([CD, 2, NTOK], BF16, tag="bcd", name="sto")
                    nc.vector.tensor_sub(sto[:, 0, :], on_h[:, 0, :], on_h[:, 3, :])
                    nc.vector.tensor_add(sto[:, 1, :], on_h[:, 1, :], on_h[:, 2, :])
                    nc.vector.tensor_sub(O[0:CD, h, :], sto[:, 0, :], sto[:, 1, :])
                    oi = nrmp.tile([CD, NTOK], BF16, tag="bcr", name="oi")
                    nc.vector.tensor_add(oi[:], sto[:, 0, :], sto[:, 1, :])
                    nc.gpsimd.dma_start(O[CD:P, h, :], oi[:])

                on_tiles = {}

                def emit_kp_head(kp, h, pre=None):
                    # kp=0: g-pair (0, 2) with vr; kp=1: (1, 3) with vi
                    if h not in on_tiles:
                        on_tiles[h] = onp.tile(
                            [CD, HL, NTOK], BF16, tag=f"on_{h}", name=f"on_{h}"
                        )
                    oavs = []
                    dens = []
                    for gi in range(2):
                        oav = oavp.tile([CD + 1, NTOK], F32, tag="oav", name="oav")
                        den0 = nrmp.tile(
                            [1, NTOK], F32, tag=f"den{gi}", name="den0", bufs=1
                        )
                        oavs.append(oav)
                        dens.append(den0)
                    for ic in range(2):
                        if pre is not None and ic in pre:
                            pts = pre[ic]
                        else:
                            pts = emit_pass_qk(h, kp, ic)
                        emit_pass_av(h, kp, ic, pts, oavs, dens)
                    for gi in range(2):
                        g = kp + 2 * gi  # kr: g0,g2 ; ki: g1,g3
                        emit_norm(h, g, oavs[gi], dens[gi], on_tiles[h])
                    if kp == 1:
                        emit_combine(h, on_tiles[h])

                # ---------------- interleaved A/B emission ----------------
                emit_qr_w()
                emit_k_w_pair("wkr", krT, 0)
                emit_k_w_pair("wkr", krT, 1)
                emit_qi_w()
                # warm-up: first kr-pass QK/exp before the v chains
                warm = {0: emit_pass_qk(0, 0, 0)}
                emit_v(0)
                emit_kp_head(0, 0, pre=warm)
                emit_kp_head(0, 1)
                emit_k_w_pair("wki", kiT, 0)
                emit_kp_head(0, 2)
                emit_k_w_pair("wki", kiT, 1)
                emit_kp_head(0, 3)
                emit_v(1)

            # A weights + xs freed here; C partial tiles reuse the space.
            with tc.tile_pool(name="pc", bufs=1) as pcp:
                ypart_tiles = {}

                def emit_c_pass(phase):
                    # output projection in two half-contractions: heads 0,1
                    # accumulate to a bf16 partial mid-attention; heads 2,3
                    # add on top at the tail.
                    for ri in range(2):
                        for tt in range(8):
                            slot = psp.tile([P, NTOK], F32, tag="s", name="sloty")
                            for oc in range(2):
                                for kt in (0, 1) if phase == 0 else (2, 3):
                                    nc.tensor.matmul(
                                        slot[:, oc * 512 : (oc + 1) * 512],
                                        O[:, kt, tt * 128 : (tt + 1) * 128],
                                        wy_sb[:, ri, kt, oc * 512 : (oc + 1) * 512],
                                        start=(kt in (0, 2)),
                                        stop=(kt in (1, 3)),
                                    )
                            if phase == 0:
                                yb = pcp.tile(
                                    [P, NTOK], BF16, tag=f"yp_{ri}_{tt}",
                                    name=f"yp_{ri}_{tt}",
                                )
                                nc.scalar.copy(yb[:], slot[:])
                                ypart_tiles[(ri, tt)] = yb
                            else:
                                ys = pcp.tile(
                                    [P, NTOK], F32, tag="ys", name="ys", bufs=3
                                )
                                nc.vector.tensor_add(
                                    ys[:], slot[:], ypart_tiles[(ri, tt)][:]
                                )
                                nc.sync.dma_start(
                                    yp[ri, tt * 128 : (tt + 1) * 128, :], ys[:]
                                )

                emit_kp_head(1, 0)
                emit_kp_head(1, 1)
                emit_c_pass(0)
                emit_kp_head(1, 2)
                emit_kp_head(1, 3)
                emit_c_pass(1)

            if dbg:
                nc.sync.dma_start(dbg["d_qT2"][:, :, :], qT2[:])
                nc.sync.dma_start(dbg["d_krT"][:, :, :], krT[:])
                nc.sync.dma_start(dbg["d_vhat"][:, :, :, :, :], vhat[:])
                nc.sync.dma_start(dbg["d_O"][:, :, :], O[:])
    nc.compile()
    return nc


def _prep(inputs):
    f = np.float32
    bf = ml_dtypes.bfloat16
    xr = np.asarray(inputs["x_real"], f)
    xi = np.asarray(inputs["x_imag"], f)
    wq_r = np.asarray(inputs["wq_r"], f)
    wq_i = np.asarray(inputs["wq_i"], f)
    wkv_r = np.asarray(inputs["wkv_r"], f)
    wkv_i = np.asarray(inputs["wkv_i"], f)
    wout_r = np.asarray(inputs["wout_r"], f)
    wout_i = np.asarray(inputs["wout_i"], f)

    c = lambda a: np.ascontiguousarray(a).astype(bf)
    # per-pair head swap ([qi_hB | qi_hA]) for the wqi evacuation dance
    swp4 = np.arange(256).reshape(2, 2, 64)[:, [1, 0], :].reshape(-1)
    in_maps = []
    for core in range(8):
        b, hg = divmod(core, 4)
        c0 = hg * 256
        X = np.concatenate([xr[b].T, xi[b].T], axis=0)
        sl = slice(c0, c0 + 256)
        vsl = slice(1024 + c0, 1024 + c0 + 256)
        wqi_full = np.concatenate([wq_i[sl].T, wq_r[sl].T], axis=0)  # [2048, 256]
        m = {
            "x": c(X),
            "wqr": c(np.concatenate([wq_r[sl].T, -wq_i[sl].T], axis=0)),
            "wqi": c(wqi_full[:, swp4]),
            "wkr": c(np.concatenate([wkv_r[sl].T, -wkv_i[sl].T], axis=0)),
            "wki": c(np.concatenate([wkv_i[sl].T, wkv_r[sl].T], axis=0)),
            "wv": c(
                np.concatenate(
                    [
                        np.concatenate([wkv_r[vsl].T, -wkv_i[vsl].T], axis=0),
                        np.concatenate([wkv_i[vsl].T, wkv_r[vsl].T], axis=0),
                    ],
                    axis=1,
                )
            ),
        }
        Wyr = np.empty((512, 1024), f)
        Wyi = np.empty((512, 1024), f)
        for h in range(HL):
            cols = slice(c0 + h * CD, c0 + (h + 1) * CD)
            Wyr[h * 128 : h * 128 + CD] = wout_r[:, cols].T
            Wyr[h * 128 + CD : (h + 1) * 128] = -wout_i[:, cols].T
            Wyi[h * 128 : h * 128 + CD] = wout_i[:, cols].T
            Wyi[h * 128 + CD : (h + 1) * 128] = wout_r[:, cols].T
        m["wyr"] = c(Wyr)
        m["wyi"] = c(Wyi)
        in_maps.append(m)
    return in_maps


def _get_nc():
    global _nc_cache
    if _nc_cache is None:
        _nc_cache = _build()
    return _nc_cache


def _assemble(results):
    y = np.zeros((2, 2, NTOK, 1024), np.float32)
    for core in range(8):
        b = core // 4
        y[:, b] += results[core]["ypart"]
    return y


def run(inputs, trace=False, **kwargs):
    nc = _get_nc()
    in_maps = _prep(inputs)
    res = run_bass_kernel_spmd(
        nc, in_maps, core_ids=list(range(8)), trace=trace, **kwargs
    )
    return _assemble(res.results), res


def kernel(**inputs) -> np.ndarray:
    y, _ = run(inputs)
    return y

